# revision 14
# baseline (speedup 1.0000x reference)
"""Contrastive-loss Bass kernel v2: bulk SWDGE gathers (InstDMAGatherAnt).

Baseline: 432 one-column INDIRECT1D DMAs (128 rows each), ~1.45 us each,
engine-serial on the Pool sequencer -> 629 us.  v2 routes the nonmatch row
gathers through the Ant dma_gather ucode in <=1024-index chunks round-robined
over the 4 SWDGE queues; descriptor generation runs on 4 Q7 core-pairs in
parallel (~2 ns/row measured vs ~8.6 engine-serialized).  The small match
list (2560 rows/core) keeps the baseline INDIRECT1D per-column path, spread
between gather dispatches so it hides in the Pool engine's slack.

Table layout per (batch, side): rows packed 4-per-256B-granule (granule =
row>>2, subrow = row&3), one zero pad granule per 32768-granule window so
int16 gather indices reach everything: table position P = g + (g>=32767) +
(g>=65534); window w = P>>15; local idx = P & 32767; window pad local idx =
(32767, 32767, 11266).  Declared [98304, 64] f32; also shipped reshaped as
[393216, 16] for the match INDIRECT1D path (row index = 4*P + subrow).

Nonmatch slot layout: pairs sorted by class (winA, winB, subA, subB) =
9 boxes x 16 sub-boxes, static caps (multiples of 128 so every sub-box is
whole gather-tile columns; slot k of a gather lands at partition k%128,
column k//128).  A-side: one bulk run per winA; B-side: one run per box;
runs cut into <=1024-idx chunks (HW SWDGE ring limit).  Pad slots gather the
window's zero granule: d = 0 contributes nothing to either accumulator.

DVE per (box, subA) unit: one strided subtract into a compact d tile — in0
reads lane offset subA*16, in1's sub-box AP dim steps (cap/128)*64 + 16 so
the subB lane offset co-varies with the sub-box column offset — then one
full-tile square and one min(., MARGIN) accumulate per round.  Host applies
hinge = MARGIN*16*25000*8 - sum(min(d^2, MARGIN)).

Falls back to the baseline kernel (kernel.py) if any sub-box count exceeds
its static cap (P ~ 1e-4 for random index lists).
"""

import numpy as np

import concourse.bacc as bacc
import concourse.tile as tile
import concourse.bass as bass
import concourse.mybir as mybir
from concourse.bass import AP, IndirectOffsetOnAxis
from concourse.library_config import mlp

B, N, D = 4, 307200, 16
M, MN = 5000, 50000
NCORES = 8
MARGIN = 0.5
M_HALF, MN_HALF = M // 2, MN // 2          # 2500 / 25000 per core

G4 = N // 4                  # 76800 real granules
WIN = 32768                  # granules per int16-addressable window
TAB_ROWS = 3 * WIN           # declared table granules (tail unused zeros)
PAD_LOCAL = (32767, 32767, 11266)
PAD_ROW = 32767 * 4          # a zero row in the [393216, 16] view
CHUNK = 1024                 # max idxs per dma_gather (HW ring limit)

M_COLS = 20                  # 128*20 = 2560 match slots per core (60 pads)
S_COLS = 3                   # 128*3 = 384 spill slots for sub-box overflow

_F32 = mybir.dt.float32
_I16 = mybir.dt.int16
_I32 = mybir.dt.int32


def _subcap(i, j, big, med, small):
    if i < 2 and j < 2:
        return big
    if i == 2 and j == 2:
        return small
    return med


NM_CAPS = [[_subcap(i, j, 384, 128, 128) for j in range(3)] for i in range(3)]


def _layout(caps):
    box_off, off = {}, 0
    for i in range(3):
        for j in range(3):
            box_off[(i, j)] = off
            off += 16 * caps[i][j]
    return box_off, off


NM_BOX_OFF, NM_TOTAL = _layout(NM_CAPS)
NM_ROUND_BASE = [NM_BOX_OFF[(i, 0)] for i in range(3)] + [NM_TOTAL]
ROUND_SLOTS = [NM_ROUND_BASE[i + 1] - NM_ROUND_BASE[i] for i in range(3)]
BUF_COLS = max(ROUND_SLOTS) // 128


def _round_gathers():
    """(round, side, win, dst_slot_off_in_round, size, idx_off) — runs cut
    into <=CHUNK pieces; idx stream = NM_A slots then NM_B slots."""
    gathers = []

    def cut(rnd, side, win, slot_start, size, round_base, stream_base):
        o = 0
        while o < size:
            n = min(CHUNK, size - o)
            gathers.append((rnd, side, win, slot_start - round_base + o, n,
                            stream_base + slot_start + o))
            o += n

    for i in range(3):
        rb = NM_ROUND_BASE[i]
        cut(i, "A", i, rb, ROUND_SLOTS[i], rb, 0)
        for j in range(3):
            cut(i, "B", j, NM_BOX_OFF[(i, j)], 16 * NM_CAPS[i][j], rb, NM_TOTAL)
    return gathers


GATHERS = _round_gathers()
N_GATHERS_ROUND = [sum(1 for g in GATHERS if g[0] == r) for r in range(3)]
# queue of the k-th gather (issue order) and cumulative per-queue counts
# through each round (for the DVE round waits; sems are per-queue because a
# semaphore may only be updated from one SWDGE queue)
GATHER_Q = [k % 4 for k in range(len(GATHERS))]
N_UNITS_ROUND = 12  # 3 boxes x 4 subA per round
QCUM = []
for r in range(3):
    cnt = [0, 0, 0, 0]
    for k, g in enumerate(GATHERS):
        if g[0] <= r:
            cnt[GATHER_Q[k]] += 1
    QCUM.append(cnt)
IDX_TOTAL = 2 * NM_TOTAL
IDX_COLS16 = IDX_TOTAL // 16

_nc_cache = None


def _build():
    nc = bacc.Bacc("TRN2", target_bir_lowering=False, debug=False,
                   num_devices=NCORES, num_swdge_queues=4)
    TA = nc.dram_tensor("TA", [TAB_ROWS, 64], _F32, kind="ExternalInput")
    TB = nc.dram_tensor("TB", [TAB_ROWS, 64], _F32, kind="ExternalInput")
    TAR = nc.dram_tensor("TAR", [TAB_ROWS * 4, 16], _F32, kind="ExternalInput")
    TBR = nc.dram_tensor("TBR", [TAB_ROWS * 4, 16], _F32, kind="ExternalInput")
    IDX = nc.dram_tensor("IDX", [128, IDX_COLS16], _I16, kind="ExternalInput")
    MIA = nc.dram_tensor("MIA", [128, M_COLS], _I32, kind="ExternalInput")
    MIB = nc.dram_tensor("MIB", [128, M_COLS], _I32, kind="ExternalInput")
    SIA = nc.dram_tensor("SIA", [128, S_COLS], _I32, kind="ExternalInput")
    SIB = nc.dram_tensor("SIB", [128, S_COLS], _I32, kind="ExternalInput")
    OUT = nc.dram_tensor("OUT", [128, 8], _F32, kind="ExternalOutput")

    WM = M_COLS * D
    # match columns all issued by end of round 1 so their queue-0 descriptors
    # drain before the DVE consumes them ahead of round-2's units
    mcuts = [0, 10, M_COLS, M_COLS]

    # per-(round, queue) gather counts for the DVE waits
    nrq = [[0] * 4 for _ in range(3)]
    for k, g in enumerate(GATHERS):
        nrq[g[0]][GATHER_Q[k]] += 1

    from contextlib import ExitStack
    with ExitStack() as _st:
        block = _st.enter_context(nc.Block())
        sb = lambda *a: _st.enter_context(nc.sbuf_tensor(*a))
        sem = lambda n: _st.enter_context(nc.semaphore(n))
        idx_sb = sb("idx_sb", [128, IDX_COLS16], _I16)
        mia_sb = sb("mia_sb", [128, M_COLS], _I32)
        mib_sb = sb("mib_sb", [128, M_COLS], _I32)
        gA0 = sb("gA0", [128, BUF_COLS, 64], _F32)
        gA1 = sb("gA1", [128, BUF_COLS, 64], _F32)
        gB0 = sb("gB0", [128, BUF_COLS, 64], _F32)
        gB1 = sb("gB1", [128, BUF_COLS, 64], _F32)
        dt = sb("dt", [128, BUF_COLS * 16], _F32)
        mga = sb("mga", [128, WM], _F32)
        mgb = sb("mgb", [128, WM], _F32)
        sia_sb = sb("sia_sb", [128, S_COLS], _I32)
        sib_sb = sb("sib_sb", [128, S_COLS], _I32)
        sga = sb("sga", [128, S_COLS * D], _F32)
        sgb = sb("sgb", [128, S_COLS * D], _F32)
        res = sb("res", [128, 8], _F32)
        io = sem("io")
        qs = [[sem(f"q{r}_{qn}") for qn in range(4)] for r in range(3)]
        msems = [sem(f"m{r}") for r in range(3)]
        ssem = sem("ss")
        rel0 = sem("rel0")
        fin = sem("fin")

        gbufs = [(gA0, gB0), (gA1, gB1)]

        @block.sync
        def _(sy):
            # input loads on HWDGE: off the saturated Pool engine, and HWDGE
            # descriptors don't contend for the SWDGE descriptor rings
            sy.dma_start(idx_sb[:], IDX[:]).then_inc(io, 16)
            sy.dma_start(mia_sb[:], MIA[:]).then_inc(io, 16)
            sy.dma_start(mib_sb[:], MIB[:]).then_inc(io, 16)
            sy.dma_start(sia_sb[:], SIA[:]).then_inc(io, 16)
            sy.dma_start(sib_sb[:], SIB[:]).then_inc(io, 16)

        @block.gpsimd
        def _(gp: bass.BassGpSimd):
            gp.load_library(mlp)
            gp.wait_ge(io, 80)  # all inputs resident (idx_sb read at desc-gen)
            idx_ready = [True]

            def all_loads_ready():
                if not idx_ready[0]:
                    gp.wait_ge(io, 80)
                    idx_ready[0] = True
            for r in range(3):
                if r == 2:
                    gp.wait_ge(rel0, 1)
                bufA, bufB = gbufs[r % 2]
                def emit_gathers():
                    for k, (rr, side, win, dst_off, size, idx_off) in enumerate(GATHERS):
                        if rr != r:
                            continue
                        src = TA if side == "A" else TB
                        buf = bufA if side == "A" else bufB
                        qn = GATHER_Q[k]
                        gp.dma_gather(
                            buf[:, dst_off // 128:(dst_off + size) // 128, :],
                            src[win * WIN:(win + 1) * WIN, :],
                            idx_sb[:, idx_off // 16:(idx_off + size) // 16],
                            size, size, 64,
                            queue_num=qn,
                        ).then_inc(qs[r][qn], 16)
                if r != 2:
                    emit_gathers()
                if r == 1:
                    for jc in range(S_COLS):
                        gp.indirect_dma_start(
                            out=sga[:, jc * D:(jc + 1) * D], out_offset=None,
                            in_=TAR[:],
                            in_offset=IndirectOffsetOnAxis(
                                ap=sia_sb[:, jc:jc + 1], axis=0),
                        ).then_inc(ssem, 16)
                        gp.indirect_dma_start(
                            out=sgb[:, jc * D:(jc + 1) * D], out_offset=None,
                            in_=TBR[:],
                            in_offset=IndirectOffsetOnAxis(
                                ap=sib_sb[:, jc:jc + 1], axis=0),
                        ).then_inc(ssem, 16)
                if mcuts[r + 1] > mcuts[r]:
                    all_loads_ready()
                for jc in range(mcuts[r], mcuts[r + 1]):
                    gp.indirect_dma_start(
                        out=mga[:, jc * D:(jc + 1) * D], out_offset=None,
                        in_=TAR[:],
                        in_offset=IndirectOffsetOnAxis(
                            ap=mia_sb[:, jc:jc + 1], axis=0),
                    ).then_inc(msems[r], 16)
                    gp.indirect_dma_start(
                        out=mgb[:, jc * D:(jc + 1) * D], out_offset=None,
                        in_=TBR[:],
                        in_offset=IndirectOffsetOnAxis(
                            ap=mib_sb[:, jc:jc + 1], axis=0),
                    ).then_inc(msems[r], 16)
                if r == 2:
                    emit_gathers()
            gp.wait_ge(fin, 1)
            gp.dma_start(OUT[:], res[:]).then_inc(io, 16)
            gp.wait_ge(io, 96)

        @block.vector
        def _(ve):
            ve.memset(res[:], 0.0)
            part_step = BUF_COLS * 64

            def match_spill():
                # runs while round-2 gathers drain: match/spill DMAs were all
                # issued by round 1, so their queue-0 descriptors are done
                for rr in range(3):
                    if mcuts[rr + 1] > mcuts[rr]:
                        ve.wait_ge(msems[rr],
                                   16 * 2 * (mcuts[rr + 1] - mcuts[rr]))
                ve.wait_ge(ssem, 16 * 2 * S_COLS)
                ve.drain()
                ve.tensor_tensor(out=dt[:, 0:WM], in0=mga[:], in1=mgb[:],
                                 op=mybir.AluOpType.subtract)
                ve.tensor_tensor(out=dt[:, WM:WM + S_COLS * D], in0=sga[:],
                                 in1=sgb[:], op=mybir.AluOpType.subtract)
                ve.drain()
                md = dt[:, 0:WM]
                ve.scalar_tensor_tensor(
                    out=mga[:], in0=md, scalar=0.0, in1=md,
                    op0=mybir.AluOpType.add, op1=mybir.AluOpType.mult,
                    accum_out=res[:, 3:4],
                )
                sd = dt[:, WM:WM + S_COLS * D]
                ve.tensor_tensor(out=sga[:], in0=sd, in1=sd,
                                 op=mybir.AluOpType.mult)
                ve.drain()
                ve.tensor_scalar(
                    out=sgb[:], in0=sga[:], scalar1=MARGIN, scalar2=None,
                    op0=mybir.AluOpType.min, op1=mybir.AluOpType.add,
                    accum_out=res[:, 4:5],
                )

            for r in range(3):
                if r == 2:
                    match_spill()
                for qn in range(4):
                    if nrq[r][qn]:
                        ve.wait_ge(qs[r][qn], 16 * nrq[r][qn])
                if r >= 1:
                    ve.drain()   # units write dt read by prior round's tail
                bufA, bufB = gbufs[r % 2]
                base = NM_ROUND_BASE[r]
                dt_off = 0
                for j in range(3):
                    cap = NM_CAPS[r][j]
                    ccols = cap // 128
                    for sA in range(4):
                        u_cols = (NM_BOX_OFF[(r, j)] - base) // 128 + sA * 4 * ccols
                        n_el = 4 * ccols * 16
                        a_ap = AP(bufA, u_cols * 64 + sA * 16,
                                  [[part_step, 128], [ccols * 64, 4],
                                   [64, ccols], [1, 16]])
                        b_ap = AP(bufB, u_cols * 64,
                                  [[part_step, 128], [ccols * 64 + 16, 4],
                                   [64, ccols], [1, 16]])
                        d_ap = AP(dt, dt_off, [[BUF_COLS * 16, 128], [1, n_el]])
                        ve.tensor_tensor(out=d_ap, in0=a_ap, in1=b_ap,
                                         op=mybir.AluOpType.subtract)
                        dt_off += n_el
                ve.drain()
                full = dt[:, :dt_off]
                inst = ve.tensor_tensor(out=full, in0=full, in1=full,
                                        op=mybir.AluOpType.mult)
                if r == 0:
                    inst.then_inc(rel0, 1)
                ve.drain()
                inst2 = ve.tensor_scalar(
                    out=full, in0=full, scalar1=MARGIN, scalar2=None,
                    op0=mybir.AluOpType.min, op1=mybir.AluOpType.add,
                    accum_out=res[:, r:r + 1],
                )
                if r == 2:
                    inst2.then_inc(fin, 1)

    nc.compile()
    return nc


def _get_nc():
    global _nc_cache
    if _nc_cache is None:
        _nc_cache = _build()
    return _nc_cache


def _build_table(X):
    X4 = np.ascontiguousarray(X, dtype=np.float32).reshape(G4, 64)
    T = np.zeros((TAB_ROWS, 64), np.float32)
    T[0:32767] = X4[0:32767]
    T[32768:65535] = X4[32767:65534]
    T[65536:65536 + (G4 - 65534)] = X4[65534:G4]
    return T


def _map_rows(r):
    g = r >> 2
    s = (r & 3).astype(np.int64)
    P = g + (g >= 32767) + (g >= 65534)
    w = (P >> 15).astype(np.int64)
    return w, (P & 32767).astype(np.int64), s, P


def _pack_nm(iA, iB):
    """-> (idx_A[NM_TOTAL], idx_B[NM_TOTAL], spill_rows_A, spill_rows_B);
    None if the spill area overflows."""
    wA, lA, sA, PA = _map_rows(iA)
    wB, lB, sB, PB = _map_rows(iB)
    rowsA = 4 * PA + sA
    rowsB = 4 * PB + sB
    cls = ((wA * 3 + wB) * 4 + sA) * 4 + sB
    order = np.argsort(cls, kind="stable")
    counts = np.bincount(cls, minlength=144)

    idx_A = np.empty(NM_TOTAL, np.int64)
    idx_B = np.empty(NM_TOTAL, np.int64)
    for i in range(3):
        for j in range(3):
            o = NM_BOX_OFF[(i, j)]
            sz = 16 * NM_CAPS[i][j]
            idx_A[o:o + sz] = PAD_LOCAL[i]
            idx_B[o:o + sz] = PAD_LOCAL[j]
    pos = 0
    spillA, spillB = [], []
    for c in range(144):
        n = int(counts[c])
        if n == 0:
            continue
        box, sub = divmod(c, 16)
        i, j = divmod(box, 3)
        sA_, sB_ = divmod(sub, 4)
        cap = NM_CAPS[i][j]
        sl = order[pos:pos + n]
        pos += n
        k = min(n, cap)
        o = NM_BOX_OFF[(i, j)] + (sA_ * 4 + sB_) * cap
        idx_A[o:o + k] = lA[sl[:k]]
        idx_B[o:o + k] = lB[sl[:k]]
        if n > cap:
            spillA.append(rowsA[sl[cap:]])
            spillB.append(rowsB[sl[cap:]])
    spillA = np.concatenate(spillA) if spillA else np.empty(0, np.int64)
    spillB = np.concatenate(spillB) if spillB else np.empty(0, np.int64)
    if spillA.size > 128 * S_COLS:
        return None
    return idx_A, idx_B, spillA, spillB


def _pack_idx_tensor(idx_A, idx_B):
    flat = np.concatenate([idx_A, idx_B])
    blk = flat.reshape(-1, 16).T.astype(np.int16)
    return np.ascontiguousarray(np.tile(blk, (8, 1)))


def _pack_rows(rows, ncols):
    """table-row indices -> [128, ncols] int32, padded with PAD_ROW."""
    flat = np.full(128 * ncols, PAD_ROW, dtype=np.int32)
    flat[:rows.size] = rows.astype(np.int32)
    return flat.reshape(128, ncols)


def _match_rows(idx):
    _, _, s, P = _map_rows(idx)
    return 4 * P + s


def make_in_maps(inputs):
    outA = np.asarray(inputs["outA"], dtype=np.float32)
    outB = np.asarray(inputs["outB"], dtype=np.float32)
    matchA = np.asarray(inputs["matchA"]).astype(np.int64)
    matchB = np.asarray(inputs["matchB"]).astype(np.int64)
    nonMatchA = np.asarray(inputs["nonMatchA"]).astype(np.int64)
    nonMatchB = np.asarray(inputs["nonMatchB"]).astype(np.int64)

    TAs = [_build_table(outA[b]) for b in range(B)]
    TBs = [_build_table(outB[b]) for b in range(B)]

    in_maps = []
    for c in range(NCORES):
        b, h = divmod(c, 2)
        msl = slice(h * M_HALF, (h + 1) * M_HALF)
        nsl = slice(h * MN_HALF, (h + 1) * MN_HALF)
        nm = _pack_nm(nonMatchA[b, nsl], nonMatchB[b, nsl])
        if nm is None:
            return None
        idx_A, idx_B, spillA, spillB = nm
        in_maps.append({
            "TA": TAs[b], "TB": TBs[b],
            "TAR": TAs[b].reshape(TAB_ROWS * 4, 16),
            "TBR": TBs[b].reshape(TAB_ROWS * 4, 16),
            "IDX": _pack_idx_tensor(idx_A, idx_B),
            "MIA": _pack_rows(_match_rows(matchA[b, msl]), M_COLS),
            "MIB": _pack_rows(_match_rows(matchB[b, msl]), M_COLS),
            "SIA": _pack_rows(spillA, S_COLS),
            "SIB": _pack_rows(spillB, S_COLS),
        })
    return in_maps


def reduce_results(results):
    m_sum = 0.0
    nm_clip = 0.0
    for c in range(NCORES):
        r = np.asarray(results[c]["OUT"], dtype=np.float64)
        nm_clip += r[:, 0:3].sum() + r[:, 4].sum()
        m_sum += r[:, 3].sum()
    hinge = MARGIN * (D * MN_HALF) * NCORES - nm_clip
    matchLossSum = np.float32(m_sum / M)
    nonMatchLossSum = np.float32(hinge / MN)
    contrastiveLossSum = np.float32(matchLossSum + nonMatchLossSum)
    return (contrastiveLossSum, matchLossSum, nonMatchLossSum)


def run(inputs, trace=False):
    from concourse.bass_utils import run_bass_kernel_spmd

    in_maps = make_in_maps(inputs)
    if in_maps is None:
        return _bl_run(inputs, trace=trace)
    nc = _get_nc()
    r = run_bass_kernel_spmd(nc, in_maps, list(range(NCORES)), trace=trace)
    global _last_r
    _last_r = r
    out = reduce_results(r.results)
    ns = r.exec_time_ns
    if ns is None and r.mean_exec_time_ns is not None:
        ns = int(r.mean_exec_time_ns)
    return out, ns


def kernel(**inputs):
    result, _ = run(inputs, trace=False)
    return result


# ---- inlined slow-but-always-correct fallback (original baseline) ----

_BL_B, _BL_N, _BL_D = 4, 307200, 16
_BL_M, _BL_MN = 5000, 50000
_BL_NCORES = 8
_BL_MARGIN = 0.5
_BL_NON_MATCH_W = 1.0
_BL_BIG = 1.0e3
_BL_NPAD = _BL_N + 2
_BL_M_HALF, _BL_MN_HALF = _BL_M // 2, _BL_MN // 2
_BL_M_COLS = 20
_BL_NM_COLS = 196
_BL_NM_CHUNKS = 4
_BL_NM_CCOLS = _BL_NM_COLS // _BL_NM_CHUNKS
_BL_OUT_COLS = _BL_NM_CHUNKS + 1

_bl_nc_cache = None


def _bl_build():
    nc = bacc.Bacc("TRN2", target_bir_lowering=False, debug=False, num_devices=_BL_NCORES)
    A = nc.dram_tensor("A", [_BL_NPAD, _BL_D], _F32, kind="ExternalInput")
    Bv = nc.dram_tensor("Bv", [_BL_NPAD, _BL_D], _F32, kind="ExternalInput")
    miA = nc.dram_tensor("miA", [128, _BL_M_COLS], _I32, kind="ExternalInput")
    miB = nc.dram_tensor("miB", [128, _BL_M_COLS], _I32, kind="ExternalInput")
    niA = nc.dram_tensor("niA", [128, _BL_NM_COLS], _I32, kind="ExternalInput")
    niB = nc.dram_tensor("niB", [128, _BL_NM_COLS], _I32, kind="ExternalInput")
    out = nc.dram_tensor("out", [128, _BL_OUT_COLS], _F32, kind="ExternalOutput")

    with tile.TileContext(nc) as tc:
        with (
            tc.tile_pool(name="idx", bufs=1) as idxp,
            tc.tile_pool(name="gat", bufs=2) as gatp,
            tc.tile_pool(name="tmp", bufs=2) as tmpp,
            tc.tile_pool(name="res", bufs=1) as resp,
        ):
            niA_t = idxp.tile([128, _BL_NM_COLS], _I32, tag="ia")
            niB_t = idxp.tile([128, _BL_NM_COLS], _I32, tag="ib")
            miA_t = idxp.tile([128, _BL_M_COLS], _I32, tag="ma")
            miB_t = idxp.tile([128, _BL_M_COLS], _I32, tag="mb")
            nc.sync.dma_start(out=niA_t[:], in_=niA[:])
            nc.sync.dma_start(out=niB_t[:], in_=niB[:])
            nc.sync.dma_start(out=miA_t[:], in_=miA[:])
            nc.sync.dma_start(out=miB_t[:], in_=miB[:])

            res_t = resp.tile([128, _BL_OUT_COLS], _F32)

            # nonmatch: res[:, c] = sum_free min((a-b)^2, _BL_MARGIN), chunked so
            # gather tiles double-buffer and the SWDGE ring never overfills.
            W = _BL_NM_CCOLS * _BL_D
            for c in range(_BL_NM_CHUNKS):
                ga = gatp.tile([128, W], _F32, tag="ga")
                gb = gatp.tile([128, W], _F32, tag="gb")
                for j in range(_BL_NM_CCOLS):
                    col = c * _BL_NM_CCOLS + j
                    nc.gpsimd.indirect_dma_start(
                        out=ga[:, j * _BL_D : (j + 1) * _BL_D], out_offset=None, in_=A[:],
                        in_offset=IndirectOffsetOnAxis(ap=niA_t[:, col : col + 1], axis=0),
                    )
                    nc.gpsimd.indirect_dma_start(
                        out=gb[:, j * _BL_D : (j + 1) * _BL_D], out_offset=None, in_=Bv[:],
                        in_offset=IndirectOffsetOnAxis(ap=niB_t[:, col : col + 1], axis=0),
                    )
                d_t = tmpp.tile([128, W], _F32, tag="d")
                nc.vector.tensor_tensor(
                    out=d_t[:], in0=ga[:], in1=gb[:], op=mybir.AluOpType.subtract
                )
                sq_t = tmpp.tile([128, W], _F32, tag="sq")
                nc.vector.tensor_tensor(
                    out=sq_t[:], in0=d_t[:], in1=d_t[:], op=mybir.AluOpType.mult
                )
                junk_t = tmpp.tile([128, W], _F32, tag="junk")
                nc.vector.tensor_scalar(
                    out=junk_t[:], in0=sq_t[:],
                    scalar1=_BL_MARGIN, scalar2=None, op0=mybir.AluOpType.min,
                    op1=mybir.AluOpType.add,
                    accum_out=res_t[:, c : c + 1],
                )

            # match: res[:, _BL_NM_CHUNKS] = sum_free (a-b)^2
            WM = _BL_M_COLS * _BL_D
            mga = gatp.tile([128, WM], _F32, tag="mga")
            mgb = gatp.tile([128, WM], _F32, tag="mgb")
            for j in range(_BL_M_COLS):
                nc.gpsimd.indirect_dma_start(
                    out=mga[:, j * _BL_D : (j + 1) * _BL_D], out_offset=None, in_=A[:],
                    in_offset=IndirectOffsetOnAxis(ap=miA_t[:, j : j + 1], axis=0),
                )
                nc.gpsimd.indirect_dma_start(
                    out=mgb[:, j * _BL_D : (j + 1) * _BL_D], out_offset=None, in_=Bv[:],
                    in_offset=IndirectOffsetOnAxis(ap=miB_t[:, j : j + 1], axis=0),
                )
            md_t = tmpp.tile([128, WM], _F32, tag="md")
            nc.vector.tensor_tensor(
                out=md_t[:], in0=mga[:], in1=mgb[:], op=mybir.AluOpType.subtract
            )
            msq_t = tmpp.tile([128, WM], _F32, tag="msq")
            nc.vector.scalar_tensor_tensor(
                out=msq_t[:], in0=md_t[:], scalar=0.0, in1=md_t[:],
                op0=mybir.AluOpType.add, op1=mybir.AluOpType.mult,
                accum_out=res_t[:, _BL_NM_CHUNKS : _BL_NM_CHUNKS + 1],
            )

            nc.sync.dma_start(out=out[:], in_=res_t[:])
    nc.compile()
    return nc


def _bl_get_nc():
    global _bl_nc_cache
    if _bl_nc_cache is None:
        _bl_nc_cache = _bl_build()
    return _bl_nc_cache


def _bl_pack_idx(idx, ncols, pad_value):
    flat = np.full(128 * ncols, pad_value, dtype=np.int32)
    flat[: idx.size] = idx.astype(np.int32, copy=False)
    return flat.reshape(128, ncols)


def _bl_make_in_maps(outA, outB, matchA, matchB, nonMatchA, nonMatchB):
    pad_zero = np.zeros((1, _BL_D), np.float32)
    pad_big = np.full((1, _BL_D), _BL_BIG, np.float32)
    in_maps = []
    for c in range(_BL_NCORES):
        b, h = divmod(c, 2)
        msl = slice(h * _BL_M_HALF, (h + 1) * _BL_M_HALF)
        nsl = slice(h * _BL_MN_HALF, (h + 1) * _BL_MN_HALF)
        in_maps.append(
            {
                "A": np.ascontiguousarray(
                    np.concatenate([outA[b], pad_zero, pad_big], axis=0)
                ),
                "Bv": np.ascontiguousarray(
                    np.concatenate([outB[b], pad_zero, pad_zero], axis=0)
                ),
                # match pads -> (_BL_N, _BL_N): zero rows both sides, zero contribution
                "miA": _bl_pack_idx(matchA[b, msl], _BL_M_COLS, _BL_N),
                "miB": _bl_pack_idx(matchB[b, msl], _BL_M_COLS, _BL_N),
                # nonmatch pads -> (_BL_N+1, _BL_N): d = _BL_BIG, min(d^2, _BL_MARGIN) = _BL_MARGIN cancels
                "niA": _bl_pack_idx(nonMatchA[b, nsl], _BL_NM_COLS, _BL_N + 1),
                "niB": _bl_pack_idx(nonMatchB[b, nsl], _BL_NM_COLS, _BL_N),
            }
        )
    return in_maps


def _bl_reduce_results(results):
    m_sum = 0.0
    nm_clip_sum = 0.0
    for c in range(_BL_NCORES):
        res = np.asarray(results[c]["out"], dtype=np.float64)
        nm_clip_sum += res[:, :_BL_NM_CHUNKS].sum()
        m_sum += res[:, _BL_NM_CHUNKS].sum()
    # pads contribute exactly _BL_MARGIN per element to the clip sum; the identity
    # below cancels them: sum(relu(_BL_M - d^2)) = _BL_M*K_slots - sum(min(d^2, _BL_M))
    hinge_sum = _BL_MARGIN * (128 * _BL_NM_COLS * _BL_D) * _BL_NCORES - nm_clip_sum
    matchLossSum = np.float32(m_sum / _BL_M)
    nonMatchLossSum = np.float32(_BL_NON_MATCH_W * hinge_sum / _BL_MN)
    contrastiveLossSum = np.float32(matchLossSum + nonMatchLossSum)
    return (contrastiveLossSum, matchLossSum, nonMatchLossSum)


def _bl_run(inputs, trace=False):
    """Run on the 8 NeuronCores. Returns (result_tuple, exec_time_ns_or_None)."""
    from concourse.bass_utils import run_bass_kernel_spmd

    outA = np.asarray(inputs["outA"], dtype=np.float32)
    outB = np.asarray(inputs["outB"], dtype=np.float32)
    matchA = np.asarray(inputs["matchA"])
    matchB = np.asarray(inputs["matchB"])
    nonMatchA = np.asarray(inputs["nonMatchA"])
    nonMatchB = np.asarray(inputs["nonMatchB"])

    in_maps = _bl_make_in_maps(outA, outB, matchA, matchB, nonMatchA, nonMatchB)
    nc = _bl_get_nc()
    r = run_bass_kernel_spmd(nc, in_maps, list(range(_BL_NCORES)), trace=trace)
    out = _bl_reduce_results(r.results)
    ns = r.exec_time_ns
    if ns is None and r.mean_exec_time_ns is not None:
        ns = int(r.mean_exec_time_ns)
    return out, ns






# revision 16
# speedup vs baseline: 1.0388x; 1.0388x over previous
"""Contrastive-loss Bass kernel v2: bulk SWDGE gathers (InstDMAGatherAnt).

Baseline: 432 one-column INDIRECT1D DMAs (128 rows each), ~1.45 us each,
engine-serial on the Pool sequencer -> 629 us.  v2 routes the nonmatch row
gathers through the Ant dma_gather ucode in <=1024-index chunks round-robined
over the 4 SWDGE queues; descriptor generation runs on 4 Q7 core-pairs in
parallel (~2 ns/row measured vs ~8.6 engine-serialized).  The small match
list (2560 rows/core) keeps the baseline INDIRECT1D per-column path, spread
between gather dispatches so it hides in the Pool engine's slack.

Table layout per (batch, side): rows packed 4-per-256B-granule (granule =
row>>2, subrow = row&3), one zero pad granule per 32768-granule window so
int16 gather indices reach everything: table position P = g + (g>=32767) +
(g>=65534); window w = P>>15; local idx = P & 32767; window pad local idx =
(32767, 32767, 11266).  Declared [98304, 64] f32; also shipped reshaped as
[393216, 16] for the match INDIRECT1D path (row index = 4*P + subrow).

Nonmatch slot layout: pairs sorted by class (winA, winB, subA, subB) =
9 boxes x 16 sub-boxes, static caps (multiples of 128 so every sub-box is
whole gather-tile columns; slot k of a gather lands at partition k%128,
column k//128).  A-side: one bulk run per winA; B-side: one run per box;
runs cut into <=1024-idx chunks (HW SWDGE ring limit).  Pad slots gather the
window's zero granule: d = 0 contributes nothing to either accumulator.

DVE per (box, subA) unit: one strided subtract into a compact d tile — in0
reads lane offset subA*16, in1's sub-box AP dim steps (cap/128)*64 + 16 so
the subB lane offset co-varies with the sub-box column offset — then one
full-tile square and one min(., MARGIN) accumulate per round.  Host applies
hinge = MARGIN*16*25000*8 - sum(min(d^2, MARGIN)).

Falls back to the baseline kernel (kernel.py) if any sub-box count exceeds
its static cap (P ~ 1e-4 for random index lists).
"""

import numpy as np

import concourse.bacc as bacc
import concourse.tile as tile
import concourse.bass as bass
import concourse.mybir as mybir
from concourse.bass import AP, IndirectOffsetOnAxis
from concourse.library_config import mlp

B, N, D = 4, 307200, 16
M, MN = 5000, 50000
NCORES = 8
MARGIN = 0.5
M_HALF, MN_HALF = M // 2, MN // 2          # 2500 / 25000 per core

G4 = N // 4                  # 76800 real granules
WIN = 32768                  # granules per int16-addressable window
TAB_ROWS = 3 * WIN           # declared table granules (tail unused zeros)
PAD_LOCAL = (32767, 32767, 11266)
PAD_ROW = 32767 * 4          # a zero row in the [393216, 16] view
CHUNK = 1024                 # max idxs per dma_gather (HW ring limit)

M_COLS = 20                  # 128*20 = 2560 match slots per core (60 pads)
S_COLS = 3                   # 128*3 = 384 spill slots for sub-box overflow

_F32 = mybir.dt.float32
_I16 = mybir.dt.int16
_I32 = mybir.dt.int32


def _subcap(i, j, big, med, small):
    if i < 2 and j < 2:
        return big
    if i == 2 and j == 2:
        return small
    return med


NM_CAPS = [[_subcap(i, j, 384, 128, 128) for j in range(3)] for i in range(3)]


def _layout(caps):
    box_off, off = {}, 0
    for i in range(3):
        for j in range(3):
            box_off[(i, j)] = off
            off += 16 * caps[i][j]
    return box_off, off


NM_BOX_OFF, NM_TOTAL = _layout(NM_CAPS)
NM_ROUND_BASE = [NM_BOX_OFF[(i, 0)] for i in range(3)] + [NM_TOTAL]
ROUND_SLOTS = [NM_ROUND_BASE[i + 1] - NM_ROUND_BASE[i] for i in range(3)]
BUF_COLS = max(ROUND_SLOTS) // 128


def _round_gathers():
    """(round, side, win, dst_slot_off_in_round, size, idx_off) — runs cut
    into <=CHUNK pieces; idx stream = NM_A slots then NM_B slots."""
    gathers = []

    def cut(rnd, side, win, slot_start, size, round_base, stream_base):
        o = 0
        while o < size:
            n = min(CHUNK, size - o)
            gathers.append((rnd, side, win, slot_start - round_base + o, n,
                            stream_base + slot_start + o))
            o += n

    for i in range(3):
        rb = NM_ROUND_BASE[i]
        cut(i, "A", i, rb, ROUND_SLOTS[i], rb, 0)
        for j in range(3):
            cut(i, "B", j, NM_BOX_OFF[(i, j)], 16 * NM_CAPS[i][j], rb, NM_TOTAL)
    return gathers


GATHERS = _round_gathers()
N_GATHERS_ROUND = [sum(1 for g in GATHERS if g[0] == r) for r in range(3)]
# queue of the k-th gather (issue order) and cumulative per-queue counts
# through each round (for the DVE round waits; sems are per-queue because a
# semaphore may only be updated from one SWDGE queue)
GATHER_Q = [k % 4 for k in range(len(GATHERS))]
N_UNITS_ROUND = 12  # 3 boxes x 4 subA per round
QCUM = []
for r in range(3):
    cnt = [0, 0, 0, 0]
    for k, g in enumerate(GATHERS):
        if g[0] <= r:
            cnt[GATHER_Q[k]] += 1
    QCUM.append(cnt)
IDX_TOTAL = 2 * NM_TOTAL
IDX_COLS16 = IDX_TOTAL // 16

_nc_cache = None


def _build():
    nc = bacc.Bacc("TRN2", target_bir_lowering=False, debug=False,
                   num_devices=NCORES, num_swdge_queues=4)
    TA = nc.dram_tensor("TA", [TAB_ROWS, 64], _F32, kind="ExternalInput")
    TB = nc.dram_tensor("TB", [TAB_ROWS, 64], _F32, kind="ExternalInput")
    TAR = nc.dram_tensor("TAR", [TAB_ROWS * 4, 16], _F32, kind="ExternalInput")
    TBR = nc.dram_tensor("TBR", [TAB_ROWS * 4, 16], _F32, kind="ExternalInput")
    IDX = nc.dram_tensor("IDX", [128, IDX_COLS16], _I16, kind="ExternalInput")
    MIA = nc.dram_tensor("MIA", [128, M_COLS], _I32, kind="ExternalInput")
    MIB = nc.dram_tensor("MIB", [128, M_COLS], _I32, kind="ExternalInput")
    SIA = nc.dram_tensor("SIA", [128, S_COLS], _I32, kind="ExternalInput")
    SIB = nc.dram_tensor("SIB", [128, S_COLS], _I32, kind="ExternalInput")
    OUT = nc.dram_tensor("OUT", [128, 8], _F32, kind="ExternalOutput")

    WM = M_COLS * D
    # match columns interleaved into rounds: r0 7, r1 7, r2 6 (+ spill in r2)
    mcuts = [0, 7, 14, M_COLS]

    # per-(round, queue) gather counts for the DVE waits
    nrq = [[0] * 4 for _ in range(3)]
    for k, g in enumerate(GATHERS):
        nrq[g[0]][GATHER_Q[k]] += 1

    from contextlib import ExitStack
    with ExitStack() as _st:
        block = _st.enter_context(nc.Block())
        sb = lambda *a: _st.enter_context(nc.sbuf_tensor(*a))
        sem = lambda n: _st.enter_context(nc.semaphore(n))
        idx_sb = sb("idx_sb", [128, IDX_COLS16], _I16)
        mia_sb = sb("mia_sb", [128, M_COLS], _I32)
        mib_sb = sb("mib_sb", [128, M_COLS], _I32)
        gA0 = sb("gA0", [128, BUF_COLS, 64], _F32)
        gA1 = sb("gA1", [128, BUF_COLS, 64], _F32)
        gB0 = sb("gB0", [128, BUF_COLS, 64], _F32)
        gB1 = sb("gB1", [128, BUF_COLS, 64], _F32)
        dt = sb("dt", [128, BUF_COLS * 16], _F32)
        mga = sb("mga", [128, WM], _F32)
        mgb = sb("mgb", [128, WM], _F32)
        sia_sb = sb("sia_sb", [128, S_COLS], _I32)
        sib_sb = sb("sib_sb", [128, S_COLS], _I32)
        sga = sb("sga", [128, S_COLS * D], _F32)
        sgb = sb("sgb", [128, S_COLS * D], _F32)
        res = sb("res", [128, 8], _F32)
        io = sem("io")
        qs = [[sem(f"q{r}_{qn}") for qn in range(4)] for r in range(3)]
        msems = [sem(f"m{r}") for r in range(3)]
        ssem = sem("ss")
        rel0 = sem("rel0")
        fin = sem("fin")

        gbufs = [(gA0, gB0), (gA1, gB1)]

        @block.sync
        def _(sy):
            # input loads on HWDGE: off the saturated Pool engine, and clear
            # of the SWDGE descriptor rings — first gather descriptors reach
            # the DMA engines ~8 us earlier (ramp 17.5 -> 9.8 us measured)
            sy.dma_start(idx_sb[:], IDX[:]).then_inc(io, 16)
            sy.dma_start(mia_sb[:], MIA[:]).then_inc(io, 16)
            sy.dma_start(mib_sb[:], MIB[:]).then_inc(io, 16)
            sy.dma_start(sia_sb[:], SIA[:]).then_inc(io, 16)
            sy.dma_start(sib_sb[:], SIB[:]).then_inc(io, 16)

        @block.gpsimd
        def _(gp: bass.BassGpSimd):
            gp.load_library(mlp)
            gp.wait_ge(io, 80)
            idx_ready = [True]

            def all_loads_ready():
                if not idx_ready[0]:
                    gp.wait_ge(io, 80)
                    idx_ready[0] = True
            for r in range(3):
                if r == 2:
                    gp.wait_ge(rel0, 1)
                bufA, bufB = gbufs[r % 2]
                def emit_gathers():
                    for k, (rr, side, win, dst_off, size, idx_off) in enumerate(GATHERS):
                        if rr != r:
                            continue
                        src = TA if side == "A" else TB
                        buf = bufA if side == "A" else bufB
                        qn = GATHER_Q[k]
                        gp.dma_gather(
                            buf[:, dst_off // 128:(dst_off + size) // 128, :],
                            src[win * WIN:(win + 1) * WIN, :],
                            idx_sb[:, idx_off // 16:(idx_off + size) // 16],
                            size, size, 64,
                            queue_num=qn,
                        ).then_inc(qs[r][qn], 16)
                if r != 2:
                    emit_gathers()
                if r == 2:
                    all_loads_ready()
                    for jc in range(S_COLS):
                        gp.indirect_dma_start(
                            out=sga[:, jc * D:(jc + 1) * D], out_offset=None,
                            in_=TAR[:],
                            in_offset=IndirectOffsetOnAxis(
                                ap=sia_sb[:, jc:jc + 1], axis=0),
                        ).then_inc(ssem, 16)
                        gp.indirect_dma_start(
                            out=sgb[:, jc * D:(jc + 1) * D], out_offset=None,
                            in_=TBR[:],
                            in_offset=IndirectOffsetOnAxis(
                                ap=sib_sb[:, jc:jc + 1], axis=0),
                        ).then_inc(ssem, 16)
                if mcuts[r + 1] > mcuts[r]:
                    all_loads_ready()
                for jc in range(mcuts[r], mcuts[r + 1]):
                    gp.indirect_dma_start(
                        out=mga[:, jc * D:(jc + 1) * D], out_offset=None,
                        in_=TAR[:],
                        in_offset=IndirectOffsetOnAxis(
                            ap=mia_sb[:, jc:jc + 1], axis=0),
                    ).then_inc(msems[r], 16)
                    gp.indirect_dma_start(
                        out=mgb[:, jc * D:(jc + 1) * D], out_offset=None,
                        in_=TBR[:],
                        in_offset=IndirectOffsetOnAxis(
                            ap=mib_sb[:, jc:jc + 1], axis=0),
                    ).then_inc(msems[r], 16)
                if r == 2:
                    emit_gathers()
            gp.wait_ge(fin, 1)
            gp.dma_start(OUT[:], res[:]).then_inc(io, 16)
            gp.wait_ge(io, 96)

        @block.vector
        def _(ve):
            ve.memset(res[:], 0.0)
            part_step = BUF_COLS * 64
            for r in range(3):
                for qn in range(4):
                    if nrq[r][qn]:
                        ve.wait_ge(qs[r][qn], 16 * nrq[r][qn])
                if r >= 1:
                    ve.drain()   # units write dt read by prior round's tail
                bufA, bufB = gbufs[r % 2]
                base = NM_ROUND_BASE[r]
                dt_off = 0
                for j in range(3):
                    cap = NM_CAPS[r][j]
                    ccols = cap // 128
                    for sA in range(4):
                        u_cols = (NM_BOX_OFF[(r, j)] - base) // 128 + sA * 4 * ccols
                        n_el = 4 * ccols * 16
                        a_ap = AP(bufA, u_cols * 64 + sA * 16,
                                  [[part_step, 128], [ccols * 64, 4],
                                   [64, ccols], [1, 16]])
                        b_ap = AP(bufB, u_cols * 64,
                                  [[part_step, 128], [ccols * 64 + 16, 4],
                                   [64, ccols], [1, 16]])
                        d_ap = AP(dt, dt_off, [[BUF_COLS * 16, 128], [1, n_el]])
                        ve.tensor_tensor(out=d_ap, in0=a_ap, in1=b_ap,
                                         op=mybir.AluOpType.subtract)
                        dt_off += n_el
                ve.drain()
                full = dt[:, :dt_off]
                inst = ve.tensor_tensor(out=full, in0=full, in1=full,
                                        op=mybir.AluOpType.mult)
                if r == 0:
                    inst.then_inc(rel0, 1)
                ve.drain()
                ve.tensor_scalar(
                    out=full, in0=full, scalar1=MARGIN, scalar2=None,
                    op0=mybir.AluOpType.min, op1=mybir.AluOpType.add,
                    accum_out=res[:, r:r + 1],
                )
            # match + spill
            for r in range(3):
                ve.wait_ge(msems[r], 16 * 2 * (mcuts[r + 1] - mcuts[r]))
            ve.wait_ge(ssem, 16 * 2 * S_COLS)
            ve.drain()
            ve.tensor_tensor(out=dt[:, 0:WM], in0=mga[:], in1=mgb[:],
                             op=mybir.AluOpType.subtract)
            ve.tensor_tensor(out=dt[:, WM:WM + S_COLS * D], in0=sga[:],
                             in1=sgb[:], op=mybir.AluOpType.subtract)
            ve.drain()
            md = dt[:, 0:WM]
            ve.scalar_tensor_tensor(
                out=mga[:], in0=md, scalar=0.0, in1=md,
                op0=mybir.AluOpType.add, op1=mybir.AluOpType.mult,
                accum_out=res[:, 3:4],
            )
            sd = dt[:, WM:WM + S_COLS * D]
            ve.tensor_tensor(out=sga[:], in0=sd, in1=sd,
                             op=mybir.AluOpType.mult)
            ve.drain()
            ve.tensor_scalar(
                out=sgb[:], in0=sga[:], scalar1=MARGIN, scalar2=None,
                op0=mybir.AluOpType.min, op1=mybir.AluOpType.add,
                accum_out=res[:, 4:5],
            ).then_inc(fin, 1)

    nc.compile()
    return nc


def _get_nc():
    global _nc_cache
    if _nc_cache is None:
        _nc_cache = _build()
    return _nc_cache


def _build_table(X):
    X4 = np.ascontiguousarray(X, dtype=np.float32).reshape(G4, 64)
    T = np.zeros((TAB_ROWS, 64), np.float32)
    T[0:32767] = X4[0:32767]
    T[32768:65535] = X4[32767:65534]
    T[65536:65536 + (G4 - 65534)] = X4[65534:G4]
    return T


def _map_rows(r):
    g = r >> 2
    s = (r & 3).astype(np.int64)
    P = g + (g >= 32767) + (g >= 65534)
    w = (P >> 15).astype(np.int64)
    return w, (P & 32767).astype(np.int64), s, P


def _pack_nm(iA, iB):
    """-> (idx_A[NM_TOTAL], idx_B[NM_TOTAL], spill_rows_A, spill_rows_B);
    None if the spill area overflows."""
    wA, lA, sA, PA = _map_rows(iA)
    wB, lB, sB, PB = _map_rows(iB)
    rowsA = 4 * PA + sA
    rowsB = 4 * PB + sB
    cls = ((wA * 3 + wB) * 4 + sA) * 4 + sB
    order = np.argsort(cls, kind="stable")
    counts = np.bincount(cls, minlength=144)

    idx_A = np.empty(NM_TOTAL, np.int64)
    idx_B = np.empty(NM_TOTAL, np.int64)
    for i in range(3):
        for j in range(3):
            o = NM_BOX_OFF[(i, j)]
            sz = 16 * NM_CAPS[i][j]
            idx_A[o:o + sz] = PAD_LOCAL[i]
            idx_B[o:o + sz] = PAD_LOCAL[j]
    pos = 0
    spillA, spillB = [], []
    for c in range(144):
        n = int(counts[c])
        if n == 0:
            continue
        box, sub = divmod(c, 16)
        i, j = divmod(box, 3)
        sA_, sB_ = divmod(sub, 4)
        cap = NM_CAPS[i][j]
        sl = order[pos:pos + n]
        pos += n
        k = min(n, cap)
        o = NM_BOX_OFF[(i, j)] + (sA_ * 4 + sB_) * cap
        idx_A[o:o + k] = lA[sl[:k]]
        idx_B[o:o + k] = lB[sl[:k]]
        if n > cap:
            spillA.append(rowsA[sl[cap:]])
            spillB.append(rowsB[sl[cap:]])
    spillA = np.concatenate(spillA) if spillA else np.empty(0, np.int64)
    spillB = np.concatenate(spillB) if spillB else np.empty(0, np.int64)
    if spillA.size > 128 * S_COLS:
        return None
    return idx_A, idx_B, spillA, spillB


def _pack_idx_tensor(idx_A, idx_B):
    flat = np.concatenate([idx_A, idx_B])
    blk = flat.reshape(-1, 16).T.astype(np.int16)
    return np.ascontiguousarray(np.tile(blk, (8, 1)))


def _pack_rows(rows, ncols):
    """table-row indices -> [128, ncols] int32, padded with PAD_ROW."""
    flat = np.full(128 * ncols, PAD_ROW, dtype=np.int32)
    flat[:rows.size] = rows.astype(np.int32)
    return flat.reshape(128, ncols)


def _match_rows(idx):
    _, _, s, P = _map_rows(idx)
    return 4 * P + s


def make_in_maps(inputs):
    outA = np.asarray(inputs["outA"], dtype=np.float32)
    outB = np.asarray(inputs["outB"], dtype=np.float32)
    matchA = np.asarray(inputs["matchA"]).astype(np.int64)
    matchB = np.asarray(inputs["matchB"]).astype(np.int64)
    nonMatchA = np.asarray(inputs["nonMatchA"]).astype(np.int64)
    nonMatchB = np.asarray(inputs["nonMatchB"]).astype(np.int64)

    TAs = [_build_table(outA[b]) for b in range(B)]
    TBs = [_build_table(outB[b]) for b in range(B)]

    in_maps = []
    for c in range(NCORES):
        b, h = divmod(c, 2)
        msl = slice(h * M_HALF, (h + 1) * M_HALF)
        nsl = slice(h * MN_HALF, (h + 1) * MN_HALF)
        nm = _pack_nm(nonMatchA[b, nsl], nonMatchB[b, nsl])
        if nm is None:
            return None
        idx_A, idx_B, spillA, spillB = nm
        in_maps.append({
            "TA": TAs[b], "TB": TBs[b],
            "TAR": TAs[b].reshape(TAB_ROWS * 4, 16),
            "TBR": TBs[b].reshape(TAB_ROWS * 4, 16),
            "IDX": _pack_idx_tensor(idx_A, idx_B),
            "MIA": _pack_rows(_match_rows(matchA[b, msl]), M_COLS),
            "MIB": _pack_rows(_match_rows(matchB[b, msl]), M_COLS),
            "SIA": _pack_rows(spillA, S_COLS),
            "SIB": _pack_rows(spillB, S_COLS),
        })
    return in_maps


def reduce_results(results):
    m_sum = 0.0
    nm_clip = 0.0
    for c in range(NCORES):
        r = np.asarray(results[c]["OUT"], dtype=np.float64)
        nm_clip += r[:, 0:3].sum() + r[:, 4].sum()
        m_sum += r[:, 3].sum()
    hinge = MARGIN * (D * MN_HALF) * NCORES - nm_clip
    matchLossSum = np.float32(m_sum / M)
    nonMatchLossSum = np.float32(hinge / MN)
    contrastiveLossSum = np.float32(matchLossSum + nonMatchLossSum)
    return (contrastiveLossSum, matchLossSum, nonMatchLossSum)


def run(inputs, trace=False):
    from concourse.bass_utils import run_bass_kernel_spmd

    in_maps = make_in_maps(inputs)
    if in_maps is None:
        return _bl_run(inputs, trace=trace)
    nc = _get_nc()
    r = run_bass_kernel_spmd(nc, in_maps, list(range(NCORES)), trace=trace)
    global _last_r
    _last_r = r
    out = reduce_results(r.results)
    ns = r.exec_time_ns
    if ns is None and r.mean_exec_time_ns is not None:
        ns = int(r.mean_exec_time_ns)
    return out, ns


def kernel(**inputs):
    result, _ = run(inputs, trace=False)
    return result


# ---- inlined slow-but-always-correct fallback (original baseline) ----

_BL_B, _BL_N, _BL_D = 4, 307200, 16
_BL_M, _BL_MN = 5000, 50000
_BL_NCORES = 8
_BL_MARGIN = 0.5
_BL_NON_MATCH_W = 1.0
_BL_BIG = 1.0e3
_BL_NPAD = _BL_N + 2
_BL_M_HALF, _BL_MN_HALF = _BL_M // 2, _BL_MN // 2
_BL_M_COLS = 20
_BL_NM_COLS = 196
_BL_NM_CHUNKS = 4
_BL_NM_CCOLS = _BL_NM_COLS // _BL_NM_CHUNKS
_BL_OUT_COLS = _BL_NM_CHUNKS + 1

_bl_nc_cache = None


def _bl_build():
    nc = bacc.Bacc("TRN2", target_bir_lowering=False, debug=False, num_devices=_BL_NCORES)
    A = nc.dram_tensor("A", [_BL_NPAD, _BL_D], _F32, kind="ExternalInput")
    Bv = nc.dram_tensor("Bv", [_BL_NPAD, _BL_D], _F32, kind="ExternalInput")
    miA = nc.dram_tensor("miA", [128, _BL_M_COLS], _I32, kind="ExternalInput")
    miB = nc.dram_tensor("miB", [128, _BL_M_COLS], _I32, kind="ExternalInput")
    niA = nc.dram_tensor("niA", [128, _BL_NM_COLS], _I32, kind="ExternalInput")
    niB = nc.dram_tensor("niB", [128, _BL_NM_COLS], _I32, kind="ExternalInput")
    out = nc.dram_tensor("out", [128, _BL_OUT_COLS], _F32, kind="ExternalOutput")

    with tile.TileContext(nc) as tc:
        with (
            tc.tile_pool(name="idx", bufs=1) as idxp,
            tc.tile_pool(name="gat", bufs=2) as gatp,
            tc.tile_pool(name="tmp", bufs=2) as tmpp,
            tc.tile_pool(name="res", bufs=1) as resp,
        ):
            niA_t = idxp.tile([128, _BL_NM_COLS], _I32, tag="ia")
            niB_t = idxp.tile([128, _BL_NM_COLS], _I32, tag="ib")
            miA_t = idxp.tile([128, _BL_M_COLS], _I32, tag="ma")
            miB_t = idxp.tile([128, _BL_M_COLS], _I32, tag="mb")
            nc.sync.dma_start(out=niA_t[:], in_=niA[:])
            nc.sync.dma_start(out=niB_t[:], in_=niB[:])
            nc.sync.dma_start(out=miA_t[:], in_=miA[:])
            nc.sync.dma_start(out=miB_t[:], in_=miB[:])

            res_t = resp.tile([128, _BL_OUT_COLS], _F32)

            # nonmatch: res[:, c] = sum_free min((a-b)^2, _BL_MARGIN), chunked so
            # gather tiles double-buffer and the SWDGE ring never overfills.
            W = _BL_NM_CCOLS * _BL_D
            for c in range(_BL_NM_CHUNKS):
                ga = gatp.tile([128, W], _F32, tag="ga")
                gb = gatp.tile([128, W], _F32, tag="gb")
                for j in range(_BL_NM_CCOLS):
                    col = c * _BL_NM_CCOLS + j
                    nc.gpsimd.indirect_dma_start(
                        out=ga[:, j * _BL_D : (j + 1) * _BL_D], out_offset=None, in_=A[:],
                        in_offset=IndirectOffsetOnAxis(ap=niA_t[:, col : col + 1], axis=0),
                    )
                    nc.gpsimd.indirect_dma_start(
                        out=gb[:, j * _BL_D : (j + 1) * _BL_D], out_offset=None, in_=Bv[:],
                        in_offset=IndirectOffsetOnAxis(ap=niB_t[:, col : col + 1], axis=0),
                    )
                d_t = tmpp.tile([128, W], _F32, tag="d")
                nc.vector.tensor_tensor(
                    out=d_t[:], in0=ga[:], in1=gb[:], op=mybir.AluOpType.subtract
                )
                sq_t = tmpp.tile([128, W], _F32, tag="sq")
                nc.vector.tensor_tensor(
                    out=sq_t[:], in0=d_t[:], in1=d_t[:], op=mybir.AluOpType.mult
                )
                junk_t = tmpp.tile([128, W], _F32, tag="junk")
                nc.vector.tensor_scalar(
                    out=junk_t[:], in0=sq_t[:],
                    scalar1=_BL_MARGIN, scalar2=None, op0=mybir.AluOpType.min,
                    op1=mybir.AluOpType.add,
                    accum_out=res_t[:, c : c + 1],
                )

            # match: res[:, _BL_NM_CHUNKS] = sum_free (a-b)^2
            WM = _BL_M_COLS * _BL_D
            mga = gatp.tile([128, WM], _F32, tag="mga")
            mgb = gatp.tile([128, WM], _F32, tag="mgb")
            for j in range(_BL_M_COLS):
                nc.gpsimd.indirect_dma_start(
                    out=mga[:, j * _BL_D : (j + 1) * _BL_D], out_offset=None, in_=A[:],
                    in_offset=IndirectOffsetOnAxis(ap=miA_t[:, j : j + 1], axis=0),
                )
                nc.gpsimd.indirect_dma_start(
                    out=mgb[:, j * _BL_D : (j + 1) * _BL_D], out_offset=None, in_=Bv[:],
                    in_offset=IndirectOffsetOnAxis(ap=miB_t[:, j : j + 1], axis=0),
                )
            md_t = tmpp.tile([128, WM], _F32, tag="md")
            nc.vector.tensor_tensor(
                out=md_t[:], in0=mga[:], in1=mgb[:], op=mybir.AluOpType.subtract
            )
            msq_t = tmpp.tile([128, WM], _F32, tag="msq")
            nc.vector.scalar_tensor_tensor(
                out=msq_t[:], in0=md_t[:], scalar=0.0, in1=md_t[:],
                op0=mybir.AluOpType.add, op1=mybir.AluOpType.mult,
                accum_out=res_t[:, _BL_NM_CHUNKS : _BL_NM_CHUNKS + 1],
            )

            nc.sync.dma_start(out=out[:], in_=res_t[:])
    nc.compile()
    return nc


def _bl_get_nc():
    global _bl_nc_cache
    if _bl_nc_cache is None:
        _bl_nc_cache = _bl_build()
    return _bl_nc_cache


def _bl_pack_idx(idx, ncols, pad_value):
    flat = np.full(128 * ncols, pad_value, dtype=np.int32)
    flat[: idx.size] = idx.astype(np.int32, copy=False)
    return flat.reshape(128, ncols)


def _bl_make_in_maps(outA, outB, matchA, matchB, nonMatchA, nonMatchB):
    pad_zero = np.zeros((1, _BL_D), np.float32)
    pad_big = np.full((1, _BL_D), _BL_BIG, np.float32)
    in_maps = []
    for c in range(_BL_NCORES):
        b, h = divmod(c, 2)
        msl = slice(h * _BL_M_HALF, (h + 1) * _BL_M_HALF)
        nsl = slice(h * _BL_MN_HALF, (h + 1) * _BL_MN_HALF)
        in_maps.append(
            {
                "A": np.ascontiguousarray(
                    np.concatenate([outA[b], pad_zero, pad_big], axis=0)
                ),
                "Bv": np.ascontiguousarray(
                    np.concatenate([outB[b], pad_zero, pad_zero], axis=0)
                ),
                # match pads -> (_BL_N, _BL_N): zero rows both sides, zero contribution
                "miA": _bl_pack_idx(matchA[b, msl], _BL_M_COLS, _BL_N),
                "miB": _bl_pack_idx(matchB[b, msl], _BL_M_COLS, _BL_N),
                # nonmatch pads -> (_BL_N+1, _BL_N): d = _BL_BIG, min(d^2, _BL_MARGIN) = _BL_MARGIN cancels
                "niA": _bl_pack_idx(nonMatchA[b, nsl], _BL_NM_COLS, _BL_N + 1),
                "niB": _bl_pack_idx(nonMatchB[b, nsl], _BL_NM_COLS, _BL_N),
            }
        )
    return in_maps


def _bl_reduce_results(results):
    m_sum = 0.0
    nm_clip_sum = 0.0
    for c in range(_BL_NCORES):
        res = np.asarray(results[c]["out"], dtype=np.float64)
        nm_clip_sum += res[:, :_BL_NM_CHUNKS].sum()
        m_sum += res[:, _BL_NM_CHUNKS].sum()
    # pads contribute exactly _BL_MARGIN per element to the clip sum; the identity
    # below cancels them: sum(relu(_BL_M - d^2)) = _BL_M*K_slots - sum(min(d^2, _BL_M))
    hinge_sum = _BL_MARGIN * (128 * _BL_NM_COLS * _BL_D) * _BL_NCORES - nm_clip_sum
    matchLossSum = np.float32(m_sum / _BL_M)
    nonMatchLossSum = np.float32(_BL_NON_MATCH_W * hinge_sum / _BL_MN)
    contrastiveLossSum = np.float32(matchLossSum + nonMatchLossSum)
    return (contrastiveLossSum, matchLossSum, nonMatchLossSum)


def _bl_run(inputs, trace=False):
    """Run on the 8 NeuronCores. Returns (result_tuple, exec_time_ns_or_None)."""
    from concourse.bass_utils import run_bass_kernel_spmd

    outA = np.asarray(inputs["outA"], dtype=np.float32)
    outB = np.asarray(inputs["outB"], dtype=np.float32)
    matchA = np.asarray(inputs["matchA"])
    matchB = np.asarray(inputs["matchB"])
    nonMatchA = np.asarray(inputs["nonMatchA"])
    nonMatchB = np.asarray(inputs["nonMatchB"])

    in_maps = _bl_make_in_maps(outA, outB, matchA, matchB, nonMatchA, nonMatchB)
    nc = _bl_get_nc()
    r = run_bass_kernel_spmd(nc, in_maps, list(range(_BL_NCORES)), trace=trace)
    out = _bl_reduce_results(r.results)
    ns = r.exec_time_ns
    if ns is None and r.mean_exec_time_ns is not None:
        ns = int(r.mean_exec_time_ns)
    return out, ns




